# revision 1
# baseline (speedup 1.0000x reference)
"""Angular prototypical loss on 8 TRN2 NeuronCores (Bass/Tile, SPMD).

kernel(**inputs): takes FULL inputs (embeddings [65536,256] f32, labels
[65536] i32, num_classes), shards the batch across the 8 cores, runs one
SPMD Bass kernel (AllReduce of per-class prototype sums on-chip), returns
the scalar mean loss. See build() for the per-core algorithm.
"""
import numpy as np
from concourse.bass_utils import run_bass_kernel_spmd

import math

import concourse.bass as bass
import concourse.mybir as mybir
import concourse.tile as tile
import concourse.bacc as bacc

P = 128
D = 256
C = 1024
NCORES = 8
MARGIN = 0.2
INV_T = 10.0
COS_M = math.cos(MARGIN)
SIN_M = math.sin(MARGIN)
TH = math.cos(math.pi - MARGIN)

f32 = mybir.dt.float32
bf16 = mybir.dt.bfloat16
fp16 = mybir.dt.float16
i32 = mybir.dt.int32
fp8 = mybir.dt.float8e4

AF = mybir.ActivationFunctionType
OP = mybir.AluOpType


def build(nt: int = 64, group: int = 8):
    """nt: row-tiles per core (rows/core = 128*nt). group: tiles per DMA group."""
    BL = P * nt
    ng = nt // group
    assert nt % group == 0

    nc = bacc.Bacc("TRN2", target_bir_lowering=False, debug=False,
                   num_devices=NCORES)
    emb = nc.declare_dram_parameter("embeddings", [BL, D], f32, isOutput=False)
    lab = nc.declare_dram_parameter("labels", [BL], i32, isOutput=False)
    out = nc.declare_dram_parameter("out", [P, 1], f32, isOutput=True)

    emb_g = emb.ap().rearrange("(p q) d -> p q d", p=P)      # [128, nt, 256]
    lab_pn = lab.ap().rearrange("(p n) -> p n", p=P)         # [128, nt]

    with tile.TileContext(nc) as tc:
        with (
            tc.tile_pool(name="big", bufs=1) as big,
            tc.tile_pool(name="stage", bufs=2) as stage,
            tc.tile_pool(name="ohp", bufs=4) as ohp,
            tc.tile_pool(name="gat", bufs=2) as gat,
            tc.tile_pool(name="scr", bufs=2) as scr,
            tc.tile_pool(name="dram", bufs=1, space="DRAM") as dram,
        ):
            s_local = dram.tile([C, D], bf16, tag="s_local")
            s_global = dram.tile([C, D], bf16, tag="s_global",
                                 addr_space="Shared")
            shat_dram = dram.tile([C, D], bf16, tag="shat_dram")
            s_local_v = s_local.rearrange("(j p) d -> p j d", p=P)
            s_global_v = s_global.rearrange("(j p) d -> p j d", p=P)
            shat_v = shat_dram.rearrange("(j p) d -> p j d", p=P)
            # ---- persistent SBUF ----
            e_bf = big.tile([P, nt * D], bf16, tag="e_bf")
            eT = big.tile([P, nt, 2, P], bf16, tag="eT")
            sT = big.tile([P, 2, C], bf16, tag="sT")
            lab_i = big.tile([P, nt], i32, tag="lab_i")
            lab_f = big.tile([P, nt], f32, tag="lab_f")
            normsq = big.tile([P, nt], f32, tag="normsq")
            invn = big.tile([P, nt], f32, tag="invn")
            m_all = big.tile([P, nt], f32, tag="m_all")
            sumexp = big.tile([P, nt], f32, tag="sumexp")
            iota16 = big.tile([P, C], fp16, tag="iota16")

            nc.gpsimd.iota(iota16[:], pattern=[[1, C]], base=0,
                           channel_multiplier=0,
                           allow_small_or_imprecise_dtypes=True)
            nc.sync.dma_start(out=lab_i[:], in_=lab_pn)
            nc.vector.tensor_copy(lab_f[:], lab_i[:])

            # ================= Phase A =================
            # Software-pipelined groups: stats (DMA + rownorm) for group g
            # are issued before the consume stage (normalize+onehot+matmul)
            # of group g-1, so the tiny reciprocal isn't queued behind bulk
            # DVE work and engines overlap across stages.
            with tc.tile_pool(name="psA", bufs=1, space="PSUM") as psA:
                proto_ps = [psA.tile([P, D], f32, tag=f"proto{j}",
                                     name=f"proto_ps{j}")
                            for j in range(8)]
                for g in range(ng):
                    raw = stage.tile([P, group, D], f32, tag="raw")
                    nc.sync.dma_start(out=raw[:],
                                      in_=emb_g[:, g * group:(g + 1) * group, :])
                    for t in range(group):
                        n = g * group + t
                        sq = scr.tile([P, D], f32, tag="sq")
                        nc.scalar.activation(
                            sq[:], raw[:, t, :], AF.Square,
                            accum_out=normsq[:, n:n + 1])
                    tmp8 = scr.tile([P, group], f32, tag="tmp8")
                    gsl = slice(g * group, (g + 1) * group)
                    nc.vector.reciprocal(tmp8[:], normsq[:, gsl])
                    nc.scalar.sqrt(invn[:, gsl], tmp8[:])
                    for t in range(group):
                        n = g * group + t
                        e_n = e_bf[:, n * D:(n + 1) * D]
                        nc.vector.tensor_scalar(
                            e_n, raw[:, t, :], invn[:, n:n + 1], None, OP.mult)
                        oh = ohp.tile([P, C], bf16, tag="oh")
                        nc.vector.tensor_scalar(
                            oh[:], iota16[:], lab_f[:, n:n + 1], None,
                            OP.is_equal)
                        for j in range(8):
                            nc.tensor.matmul(
                                out=proto_ps[j][:],
                                lhsT=oh[:, j * P:(j + 1) * P],
                                rhs=e_n,
                                start=(n == 0), stop=(n == nt - 1))
                    # one batched xbar transpose for the whole group
                    nc.sync.dma_start_transpose(
                        out=eT[:, g * group:(g + 1) * group, :, :],
                        in_=e_bf[:, g * group * D:(g + 1) * group * D])

                # ---- proto epilogue: PSUM -> SBUF (bf16 for cheap AR) ----
                s_sb = big.tile([P, 8, D], bf16, tag="s_sb")
                for j in range(8):
                    nc.scalar.copy(s_sb[:, j, :], proto_ps[j][:])

            # ---- DRAM -> AllReduce -> back ----
            nc.sync.dma_start(out=s_local_v, in_=s_sb[:])
            nc.gpsimd.collective_compute(
                "AllReduce", OP.add,
                replica_groups=[list(range(NCORES))],
                ins=[s_local[:].opt()], outs=[s_global[:].opt()])
            s_sb2 = big.tile([P, 8, D], bf16, tag="s_sb2")
            nc.sync.dma_start(out=s_sb2[:], in_=s_global_v)

            # ---- normalize prototypes ----
            pnsq = big.tile([P, 8], f32, tag="pnsq")
            pinv = big.tile([P, 8], f32, tag="pinv")
            ptmp = big.tile([P, 8], f32, tag="ptmp")
            shat_sb = big.tile([P, 8, D], bf16, tag="shat_sb")
            for j in range(8):
                sqp = scr.tile([P, D], f32, tag="sq")
                nc.scalar.activation(
                    sqp[:], s_sb2[:, j, :], AF.Square,
                    accum_out=pnsq[:, j:j + 1])
            nc.vector.reciprocal(ptmp[:], pnsq[:])
            nc.scalar.sqrt(pinv[:], ptmp[:])
            for j in range(8):
                nc.vector.tensor_scalar(
                    shat_sb[:, j, :], s_sb2[:, j, :], pinv[:, j:j + 1],
                    None, OP.mult)
            nc.sync.dma_start(out=shat_v, in_=shat_sb[:])
            for j in range(8):
                nc.sync.dma_start_transpose(
                    out=sT[:, :, j * P:(j + 1) * P], in_=shat_sb[:, j, :])

            # ================= Phase B =================
            with tc.tile_pool(name="psB", bufs=2, space="PSUM") as psB:
                for g in range(ng):
                    Gts = []
                    for t in range(group):
                        n = g * group + t
                        Gt = gat.tile([P, D], bf16, tag=f"G{t}",
                                      name=f"G_{n}")
                        nc.gpsimd.indirect_dma_start(
                            out=Gt[:], out_offset=None,
                            in_=shat_dram[:],
                            in_offset=bass.IndirectOffsetOnAxis(
                                ap=lab_i[:, n:n + 1], axis=0))
                        Gts.append(Gt)
                    for t in range(group):
                        n = g * group + t
                        cos_ps = psB.tile([P, C], f32, tag="cos")
                        for h in range(2):
                            for c in range(2):
                                nc.tensor.matmul(
                                    out=cos_ps[:, h * 512:(h + 1) * 512],
                                    lhsT=eT[:, n, c, :],
                                    rhs=sT[:, c, h * 512:(h + 1) * 512],
                                    start=(c == 0), stop=(c == 1))
                        exps = scr.tile([P, C], bf16, tag="exps")
                        nc.scalar.activation(
                            exps[:], cos_ps[:], AF.Exp, scale=INV_T,
                            accum_out=sumexp[:, n:n + 1])
                        mdot = scr.tile([P, D], bf16, tag="mdot")
                        nc.vector.tensor_tensor(
                            mdot[:], e_bf[:, n * D:(n + 1) * D], Gts[t][:],
                            op=OP.mult)
                        nc.vector.reduce_sum(m_all[:, n:n + 1], mdot[:],
                                             axis=mybir.AxisListType.X)

            # ================= Phase B epilogue (batched [P, nt]) ========
            b1 = big.tile([P, nt], f32, tag="b1")
            b2 = big.tile([P, nt], f32, tag="b2")
            b3 = big.tile([P, nt], f32, tag="b3")
            b4 = big.tile([P, nt], f32, tag="b4")
            mask = big.tile([P, nt], mybir.dt.uint8, tag="mask")
            phi_f = big.tile([P, nt], f32, tag="phi_f")

            nc.vector.tensor_tensor(b1[:], m_all[:], m_all[:], op=OP.mult)
            nc.vector.tensor_scalar(b1[:], b1[:], -1.0, 1.0, OP.mult, OP.add)
            nc.vector.tensor_scalar_max(b1[:], b1[:], 0.0)
            nc.scalar.sqrt(b2[:], b1[:])                        # sin
            nc.vector.tensor_scalar_mul(b3[:], m_all[:], COS_M)
            nc.vector.tensor_scalar(b2[:], b2[:], -SIN_M, None, OP.mult)
            nc.vector.tensor_add(b3[:], b3[:], b2[:])           # phi
            nc.vector.tensor_scalar(mask[:], m_all[:], TH, None, OP.is_gt)
            nc.vector.tensor_scalar(b4[:], m_all[:], -MARGIN, None, OP.add)
            nc.vector.select(phi_f[:], mask[:], b3[:], b4[:])
            nc.scalar.activation(b1[:], m_all[:], AF.Exp, scale=INV_T)
            nc.scalar.activation(b2[:], phi_f[:], AF.Exp, scale=INV_T)
            nc.vector.tensor_sub(b1[:], sumexp[:], b1[:])
            nc.vector.tensor_add(b1[:], b1[:], b2[:])           # Z
            nc.scalar.activation(b2[:], b1[:], AF.Ln, scale=1.0)
            nc.vector.tensor_scalar_mul(b3[:], phi_f[:], INV_T)
            nc.vector.tensor_sub(b2[:], b2[:], b3[:])           # nll
            part = big.tile([P, 1], f32, tag="part")
            nc.vector.reduce_sum(part[:], b2[:], axis=mybir.AxisListType.X)
            nc.sync.dma_start(out=out[:], in_=part[:])

    nc.compile()
    return nc


_NC_CACHE = {}


def kernel(embeddings, labels, num_classes=None, **_ignored):
    embeddings = np.ascontiguousarray(embeddings, dtype=np.float32)
    labels = np.ascontiguousarray(labels, dtype=np.int32)
    B = embeddings.shape[0]
    BL = B // NCORES

    if "nc" not in _NC_CACHE:
        _NC_CACHE["nc"] = build()
    nc = _NC_CACHE["nc"]

    in_maps = [{"embeddings": embeddings[i * BL:(i + 1) * BL],
                "labels": labels[i * BL:(i + 1) * BL]}
               for i in range(NCORES)]
    res = run_bass_kernel_spmd(nc, in_maps, list(range(NCORES)))
    total = 0.0
    for i in range(NCORES):
        total += res.results[i]["out"].astype(np.float64).sum()
    return np.float32(total / B)

